# revision 48
# baseline (speedup 1.0000x reference)
"""HGNN conv kernel for Trainium2, data-parallel over time across 8 cores.

Per core (t = core index): out = Dv^-1/2 Gc De^-1 Gc^T Dv^-1/2 (x W + 1 b^T),
computed in factored form (L never materialized):
  Gs   = dv * Gc                      [N, E]   (dv = rsqrt(rowsum Gc))
  z^T  = x^T Gs  per 128-row bf block [BF, E]  (MM1, bf16)
  v    = z W + u0 bias^T              [E, BF]  (W-MM transposes + applies W)
  out  = dv * (Gsd^T v), Gsd = de*Gc^T [N, BF] (MM2; dv folded into evict)

Host-side (layout only): all inputs packed partition-major ([128, ...]
with each partition row one contiguous HBM run) and cast to bf16, so DMA
descriptors are 4-14KB; output written partition-major f32 and unpacked
on the host. The W-MM uses blockdiag(W,W) to transpose z AND apply W in
one pass; bias enters as the rank-1 term u0 (x) bias2 folded into the
v-evict; the output-side Dv^-1/2 is folded into the MM2 psum evict.

Scheduling: a fine gating prefix (gc k0-1 quarter + single-k x chunks
on both HWDGE rings) starts MM1 at ~12us; the first six m-tiles run on
six parallel PSUM accumulators k-by-k as x arrives (stats_ps shares a
sliced ps_small buffer so 6+2 banks fit), with the de/u0 stats matmuls
interleaved per-k; gct rides last; output is stored bf16 per nb-chunk
on both rings with the final chunk split across both engines/rings.
"""

import sys

import numpy as np

sys.path.insert(0, "/opt/trn_rl_repo")

from contextlib import ExitStack

import ml_dtypes

import concourse.bass as bass
import concourse.mybir as mybir
import concourse.tile as tile
from concourse import bacc, bass_utils
from concourse.masks import make_identity

P = 128
T = 8
B = 28          # batch entries per core
N = 1024        # nodes
E = 512         # hyperedges (256 static + 256 dynamic)
F = 64          # features
BF = B * F      # 1792
EPS = 1e-6
NT = N // P     # 8 n-tiles
ET = E // P     # 4 e-tiles
MT = BF // P    # 14 bf-tiles (2 batch entries each)
NB = 4          # output free-dim chunks
NBW = BF // NB  # 448
M0 = 6          # m-tiles run on parallel accumulators during the x load

f32 = mybir.dt.float32
f32r = mybir.dt.float32r
bf16 = mybir.dt.bfloat16
BF16 = ml_dtypes.bfloat16


def _build_nc():
    nc = bacc.Bacc("TRN2", target_bir_lowering=False, debug=False)

    xs = nc.dram_tensor("xs", [P, NT * BF], bf16, kind="ExternalInput").ap()
    gc = nc.dram_tensor("gc", [P, NT * E], bf16, kind="ExternalInput").ap()
    gct = nc.dram_tensor("gct", [P, ET * N], bf16, kind="ExternalInput").ap()
    bdw = nc.dram_tensor("bdw", [P, P], bf16, kind="ExternalInput").ap()
    b2 = nc.dram_tensor("b2", [1, P], f32, kind="ExternalInput").ap()
    os_ = nc.dram_tensor("os", [P, NT * BF], bf16, kind="ExternalOutput").ap()

    with tile.TileContext(nc) as tc, ExitStack() as ctx:
        const = ctx.enter_context(tc.tile_pool(name="const", bufs=1))
        big = ctx.enter_context(tc.tile_pool(name="big", bufs=1))
        ztp = ctx.enter_context(tc.tile_pool(name="ztp", bufs=8))
        osb = ctx.enter_context(tc.tile_pool(name="osb", bufs=3))
        ps_small = ctx.enter_context(tc.tile_pool(name="ps_small", bufs=2, space="PSUM"))
        ps_z = ctx.enter_context(tc.tile_pool(name="ps_z", bufs=2, space="PSUM"))
        ps_o = ctx.enter_context(tc.tile_pool(name="ps_o", bufs=2, space="PSUM"))
        ps_x = ctx.enter_context(tc.tile_pool(name="ps_x", bufs=2, space="PSUM"))

        # ---- input loads --------------------------------------------------
        # sync ring:   gc halves, x k2-3, x k6-7, gct j0-1
        # scalar ring: x k0-1, x k4-5, gct j2-3   (stores join later)
        gc_all = big.tile([P, NT, E], bf16, name="gc_all")
        gc_r = gc.rearrange("p (k e) -> p k e", k=NT)
        xs_all = big.tile([P, NT, BF], bf16, name="xs_all")
        xs_r = xs.rearrange("p (k m) -> p k m", k=NT)
        gct_all = big.tile([P, ET, N], bf16, name="gct_all")
        gct_r = gct.rearrange("p (j n) -> p j n", j=ET)

        nc.sync.dma_start(gc_all[:, 0:2], gc_r[:, 0:2])
        nc.scalar.dma_start(xs_all[:, 1:2], xs_r[:, 1:2])
        # x k0 split: the first sweep only reads cols 0:M0*128, so gate on
        # that slice alone (downstream cumulative arrivals are unchanged)
        nc.sync.dma_start(xs_all[:, 0:1, 0 : M0 * P], xs_r[:, 0:1, 0 : M0 * P])
        nc.sync.dma_start(xs_all[:, 0:1, M0 * P : BF], xs_r[:, 0:1, M0 * P : BF])
        nc.scalar.dma_start(gc_all[:, 2:4], gc_r[:, 2:4])
        nc.scalar.dma_start(gc_all[:, 4:8], gc_r[:, 4:8])
        nc.sync.dma_start(xs_all[:, 2:4], xs_r[:, 2:4])
        nc.scalar.dma_start(xs_all[:, 4:6], xs_r[:, 4:6])
        nc.sync.dma_start(xs_all[:, 6:8], xs_r[:, 6:8])
        nc.sync.dma_start(gct_all[:, 0:2], gct_r[:, 0:2])
        nc.scalar.dma_start(gct_all[:, 2:4], gct_r[:, 2:4])

        bdw_sb = const.tile([P, P], bf16, name="bdw_sb")
        nc.gpsimd.dma_start(bdw_sb[:], bdw)
        b2_sb = const.tile([1, P], f32, name="b2_sb")
        nc.gpsimd.dma_start(b2_sb[:], b2)
        bias_bc = const.tile([P, P], f32, name="bias_bc")
        nc.gpsimd.partition_broadcast(bias_bc[:], b2_sb[:])

        ident_f = const.tile([P, P], f32, name="ident_f")
        make_identity(nc, ident_f[:])
        ident = const.tile([P, P], f32r, name="ident")
        nc.vector.tensor_copy(ident[:], ident_f[:])

        # ---- per-k dv chain + gs scale (unblocks MM1 k-tile by k-tile) ---
        eps_col = const.tile([P, 1], f32, name="eps_col")
        rs = const.tile([P, NT, 1], f32, name="rs")
        sq = const.tile([P, NT], f32, name="sq")
        dv = const.tile([P, NT], f32, name="dv")
        gs_all = big.tile([P, NT, E], bf16, name="gs_all")
        # stats stationary: [ones | dv] per k-tile (bf16)
        onesdv = const.tile([P, NT, 2], bf16, name="onesdv")
        with tc.high_priority():
            nc.vector.memset(eps_col[:], EPS)
            nc.vector.memset(onesdv[:, :, 0:1], 1.0)
            for k in range(NT):
                # rowsum on ACT: copy with accum_out (dummy copy lands in
                # gs_all, overwritten by the scale below); DVE does only
                # the recip + scale, so neither engine head-of-line blocks
                nc.scalar.activation(
                    gs_all[:, k, :], gc_all[:, k, :],
                    mybir.ActivationFunctionType.Copy,
                    accum_out=rs[:, k],
                )
                nc.scalar.activation(
                    sq[:, k : k + 1], rs[:, k], mybir.ActivationFunctionType.Sqrt,
                    bias=eps_col[:],
                )
                nc.vector.reciprocal(dv[:, k : k + 1], sq[:, k : k + 1])
                nc.vector.tensor_scalar(
                    out=gs_all[:, k, :], in0=gc_all[:, k, :],
                    scalar1=dv[:, k : k + 1],
                    scalar2=None, op0=mybir.AluOpType.mult,
                )
                nc.scalar.copy(onesdv[:, k, 1:2], dv[:, k : k + 1])

        v_all = big.tile([P, ET, BF], bf16, name="v_all")
        stats_ps = ps_small.tile([P, ET, P], f32, name="sp")[0:2].rearrange("p a b -> p (a b)")
        stats_sb = const.tile([2, E], f32r, name="stats_sb")
        statsT = const.tile([P, ET, 2], f32, name="statsT")
        de_col = const.tile([P, ET], f32, name="de_col")
        gsd_all = big.tile([P, ET, N], bf16, name="gsd_all")
        bias_u0 = const.tile([P, ET, P], f32, name="bias_u0")

        def emit_stats_tail():
            # stats_ps rows [colsum(Gc) | colsum(Gs)] already accumulated by
            # the per-k stats matmuls interleaved into the first sweep.
            nc.scalar.copy(stats_sb[:], stats_ps)
            # transpose stats to column layout [128, ET, 2]
            for j in range(ET):
                tp = ps_small.tile([P, ET, P], f32r, name="sp")[:, 0, 0:2]
                nc.tensor.matmul(
                    tp, stats_sb[:, j * P : (j + 1) * P], ident[0:2, 0:2],
                    is_transpose=True,
                )
                nc.scalar.copy(statsT[:, j, :], tp)
            nc.vector.tensor_scalar(
                out=de_col[:], in0=statsT[:, :, 0], scalar1=EPS, scalar2=None,
                op0=mybir.AluOpType.add,
            )
            nc.vector.reciprocal(de_col[:], de_col[:])
            # Gsd = de * Gc^T ; bias_u0 = u0 (x) bias2
            for j in range(ET):
                nc.vector.tensor_scalar(
                    out=bias_u0[:, j, :], in0=bias_bc[:],
                    scalar1=statsT[:, j, 1:2], scalar2=None,
                    op0=mybir.AluOpType.mult,
                )
                nc.vector.tensor_scalar(
                    out=gsd_all[:, j, :], in0=gct_all[:, j, :],
                    scalar1=de_col[:, j : j + 1], scalar2=None,
                    op0=mybir.AluOpType.mult,
                )

        def emit_wmm_from_zt(m, zt):
            wps = ps_small.tile([P, ET, P], f32, name="sp")
            for j in range(ET):
                nc.tensor.matmul(
                    wps[:, j, :], zt[:, j * P : (j + 1) * P], bdw_sb[:],
                    start=True, stop=True,
                )
            # v = bias_u0 + zw, rounded to bf16 (one instr per m-tile)
            nc.vector.scalar_tensor_tensor(
                out=v_all[:, :, m * P : (m + 1) * P],
                in0=bias_u0[:],
                scalar=1.0,
                in1=wps[:],
                op0=mybir.AluOpType.mult,
                op1=mybir.AluOpType.add,
            )

        def emit_wmm(m, zps):
            zt = ztp.tile([P, E], bf16, name="zt")
            nc.scalar.copy(zt[:], zps[:])
            emit_wmm_from_zt(m, zt)

        # ---- MM1: first M0 m-tiles on parallel accumulators, k-by-k ------
        # per-k stats matmuls ride along; WMM for m0-3 is deferred past the
        # dense sweeps so the PE never waits on the stats tail.
        zpools = [ps_z, ps_o, ps_x]
        zps4 = [
            zpools[m // 2].tile([P, E], f32, name="zps") for m in range(M0)
        ]
        for k in range(NT):
            for m in range(M0):
                nc.tensor.matmul(
                    zps4[m][:], xs_all[:, k, m * P : (m + 1) * P],
                    gs_all[:, k, :],
                    start=(k == 0), stop=(k == NT - 1),
                )
            nc.tensor.matmul(
                stats_ps, onesdv[:, k, :], gc_all[:, k, :],
                start=(k == 0), stop=(k == NT - 1),
            )
        emit_stats_tail()
        # evict first-sweep psums early so banks free for the dense sweeps
        zt4 = []
        for m in range(M0):
            zt = ztp.tile([P, E], bf16, name="zt")
            nc.scalar.copy(zt[:], zps4[m][:])
            zt4.append(zt)

        # ---- MM1 tail: remaining m-tiles, full sweeps; the first-sweep
        # W-MMs are interleaved after the m4-m7 sweeps ---------------------
        for m in range(M0, MT):
            zps = zpools[m % 3].tile([P, E], f32, name="zps")
            for k in range(NT):
                nc.tensor.matmul(
                    zps[:], xs_all[:, k, m * P : (m + 1) * P], gs_all[:, k, :],
                    start=(k == 0), stop=(k == NT - 1),
                )
            emit_wmm(m, zps)
            if m - M0 < M0:
                emit_wmm_from_zt(m - M0, zt4[m - M0])

        # ---- MM2 + store --------------------------------------------------
        os_r = os_.rearrange("p (i m) -> p i m", i=NT)
        for i in range(NT):
            ost = osb.tile([P, BF], bf16, name="ost")
            for nb in range(NB):
                ops = zpools[nb % 3].tile([P, E], f32, name="zps")[:, 0:NBW]
                for j in range(ET):
                    nc.tensor.matmul(
                        ops[:], gsd_all[:, j, i * P : (i + 1) * P],
                        v_all[:, j, nb * NBW : (nb + 1) * NBW],
                        start=(j == 0), stop=(j == ET - 1),
                    )
                # out = dv[n] * psum (alternate engines), store per nb-chunk
                dst = ost[:, nb * NBW : (nb + 1) * NBW]
                if i == NT - 1 and nb == NB - 1:
                    # last chunk: split across both evict engines and both
                    # store rings so the serial tail is halved
                    h = NBW // 2
                    nc.vector.tensor_scalar(
                        out=ost[:, nb * NBW : nb * NBW + h], in0=ops[:, 0:h],
                        scalar1=dv[:, i : i + 1], scalar2=None,
                        op0=mybir.AluOpType.mult,
                    )
                    nc.scalar.mul(
                        ost[:, nb * NBW + h : (nb + 1) * NBW], ops[:, h:NBW],
                        dv[:, i : i + 1],
                    )
                    nc.sync.dma_start(
                        os_r[:, i, nb * NBW : nb * NBW + h],
                        ost[:, nb * NBW : nb * NBW + h],
                    )
                    nc.scalar.dma_start(
                        os_r[:, i, nb * NBW + h : (nb + 1) * NBW],
                        ost[:, nb * NBW + h : (nb + 1) * NBW],
                    )
                    continue
                if nb % 2 == 0:
                    nc.vector.tensor_scalar(
                        out=dst, in0=ops[:], scalar1=dv[:, i : i + 1],
                        scalar2=None, op0=mybir.AluOpType.mult,
                    )
                else:
                    nc.scalar.mul(dst, ops[:], dv[:, i : i + 1])
                eng = nc.sync if (i * NB + nb) % 2 == 0 else nc.scalar
                eng.dma_start(
                    os_r[:, i, nb * NBW : (nb + 1) * NBW], dst
                )

    nc.finalize()
    return nc


_NC = None


def _get_nc():
    global _NC
    if _NC is None:
        _NC = _build_nc()
    return _NC


def _in_maps(x, G, G1, weight, bias):
    x = np.ascontiguousarray(x, dtype=np.float32)
    G = np.ascontiguousarray(G, dtype=np.float32)
    G1 = np.ascontiguousarray(G1, dtype=np.float32)
    weight = np.ascontiguousarray(weight, dtype=np.float32)
    bias = np.ascontiguousarray(bias, dtype=np.float32)

    # x[t,b,n,f] -> packed [T, P, (k b f)]: partition row p holds the
    # k-tile-major concat of x[t, :, k*128+p, :] (one contiguous HBM run)
    xh = np.ascontiguousarray(
        x.reshape(T, B, NT, P, F).transpose(0, 3, 2, 1, 4)
    ).reshape(T, P, NT * BF).astype(BF16)
    # Gc = [G | G1[t]] packed as [T, P, (k e)]; transpose as [T, P, (j n)]
    gc_np = np.concatenate(
        [np.broadcast_to(G[None], (T, N, 256)), G1], axis=2
    )
    gch = np.ascontiguousarray(
        gc_np.reshape(T, NT, P, E).transpose(0, 2, 1, 3)
    ).reshape(T, P, NT * E).astype(BF16)
    gcth = np.ascontiguousarray(
        gc_np.transpose(0, 2, 1).reshape(T, ET, P, N).transpose(0, 2, 1, 3)
    ).reshape(T, P, ET * N).astype(BF16)
    # blockdiag(W, W) built on host
    bdw_h = np.zeros((P, P), dtype=BF16)
    bdw_h[:F, :F] = weight.astype(BF16)
    bdw_h[F:, F:] = weight.astype(BF16)
    b2_h = np.tile(bias, 2).reshape(1, P).astype(np.float32)

    maps = []
    for c in range(T):
        maps.append(
            {
                "xs": xh[c],
                "gc": gch[c],
                "gct": gcth[c],
                "bdw": bdw_h,
                "b2": b2_h,
            }
        )
    return maps


def kernel(x, G, G1, weight, bias):
    nc = _get_nc()
    res = bass_utils.run_bass_kernel_spmd(
        nc, _in_maps(x, G, G1, weight, bias), core_ids=list(range(T))
    )
    # os: per core [P, (i b f)] bf16 -> out[b, i*128+p, f] f32
    out = np.stack([np.asarray(r["os"]) for r in res.results], axis=0)
    return np.ascontiguousarray(
        out.reshape(T, P, NT, B, F).transpose(0, 3, 2, 1, 4)
    ).reshape(T * B, N, F).astype(np.float32)
